# revision 16
# baseline (speedup 1.0000x reference)
"""Masked attention (B=2, H=8, S=4096, D=64) on 8 Trainium2 NeuronCores.

Sharding: batch*head parallel. Core c owns flat heads {2c, 2c+1} (same batch
index b = c // 4 for both, so the [S, S] mask is shared by both heads of a
core).

v4 design (per core):

  - Q^T / K^T stored twice on the partition axis (rows 0:64 == rows 64:128,
    per head). Consecutive QK^T chunk-matmuls alternate which 64-row half
    they engage, so adjacent matmuls occupy disjoint PE row groups
    (tile_position (0,0)/(64,0) auto-derived) and stream CONCURRENTLY:
    2 chunks per 512-cycle stream slot -> ~2x QK throughput. LDWEIGHTS also
    pulls ahead freely across row groups.
  - Heads processed sequentially per q-block; the QK score pool is
    double-buffered (2 x 3 PSUM banks) so the exp drains overlap the next
    group's matmuls. The q-block's full mask slice [S_k, 512] fp16 is staged
    in SBUF once and shared by both heads.
  - Softmax exp+mask split across engines (ScalarE is otherwise the
    bottleneck: ACTIVATE is 1 elem/lane/cycle):
      * 8 of 11 k-groups: ScalarE ACT exp (PSUM->SBUF fp16) + VectorE
        tensor_mul with the {1,0} fp16 mask rows.
      * 3 of 11 k-groups: one VectorE scalar_tensor_tensor emitting
        saturate_uint16(round(score * (2^10*log2e/8) + mb)): the Schraudolph
        exp2 bit trick - those uint16 bits read back as fp16
        exp(score/8) * 2^(c/1024). c = -60 zeroes the mean (1+f)/2^f
        mismatch so these rows mix unbiased with exact-exp rows in the same
        softmax. Masked entries get mb = -60000 -> y < 0 -> saturates to
        0x0000 = +0.0. The mask tile doubles as the bias tile (host bakes
        the per-row-group encoding), so masking costs nothing extra here.
  - AV accumulated transposed-free: matmul(lhsT=[V|1] chunk [128,65],
    rhs=P^T chunk [128,512] fp16) accumulates out^T[d,q] over 32 k-chunks in
    PSUM; row 64 = softmax denominator; host divides + transposes during
    unshard.
"""

from contextlib import ExitStack

import numpy as np

import concourse.tile as tile
from concourse import bacc, mybir
from concourse.bass_utils import run_bass_kernel_spmd

B, H, S, D = 2, 8, 4096, 64
N_CORES = 8
HPC = (B * H) // N_CORES  # heads per core = 2
SCALE = 1.0 / 8.0  # 1/sqrt(D)
LOG2E = 1.4426950408889634
A_STT = float(np.float32(SCALE * 1024.0 * LOG2E))  # Schraudolph slope
MB_KEEP = 15 * 1024.0 - 60.0  # kept bias; c=-60 zeroes the mean 2^f/(1+f)
# mismatch so Schraudolph-path rows mix unbiased with exact-exp rows
MB_MASK = -60000.0  # masked: y always < 0 -> uint16 saturates to +0.0

F32 = mybir.dt.float32
F16 = mybir.dt.float16
U16 = mybir.dt.uint16

GROUP_SIZE = 3
N_CHUNKS = S // 128
GROUPS = []
_c0 = 0
while _c0 < N_CHUNKS:
    GROUPS.append((_c0, min(GROUP_SIZE, N_CHUNKS - _c0)))
    _c0 += GROUP_SIZE
# groups handled by the DVE Schraudolph path (rest go ScalarE exp).
DVE_GROUPS = (1, 5, 8)
MASK_DMA_CHUNKS = 8  # mask slice arrives in 4 DMAs of 8 k-chunks each


def build_kernel_body(tc, qT, kT, vaug, maskT, outT, s=S, hpc=HPC,
                      qb_size=512, pt_bufs=6, warm_mms=44):
    """qT, kT: [hpc, 64, s] f16; vaug: [hpc, s, 65] f16; maskT: [s, s] f16
    mixed-encoding; outT: [hpc, 65, s] f32."""
    nc = tc.nc
    n_qb = s // qb_size

    ctx = ExitStack()
    const = ctx.enter_context(tc.tile_pool(name="const", bufs=1))
    mask_pool = ctx.enter_context(tc.tile_pool(name="mask", bufs=2))
    pt_pool = ctx.enter_context(tc.tile_pool(name="pt", bufs=pt_bufs))
    out_pool = ctx.enter_context(tc.tile_pool(name="osb", bufs=2))
    psum_s_pool = ctx.enter_context(
        tc.tile_pool(name="psum_s", bufs=2, space="PSUM"))
    psum_av_pool = ctx.enter_context(
        tc.tile_pool(name="psum_av", bufs=hpc, space="PSUM"))

    qT_sb = const.tile([128, hpc, s], F16)
    kT_sb = const.tile([128, hpc, s], F16)
    vaug_sb = const.tile([128, hpc, N_CHUNKS, D + 1], F16)

    def mask_dma(mk_t, qs):
        for i in range(0, N_CHUNKS, MASK_DMA_CHUNKS):
            nc.sync.dma_start(
                out=mk_t[:, i:i + MASK_DMA_CHUNKS, :],
                in_=maskT[i * 128:(i + MASK_DMA_CHUNKS) * 128, qs].rearrange(
                    "(c p) q -> p c q", p=128
                ),
            )

    # Prologue DMAs, smallest-first so compute starts early: both dup halves
    # of head0's K/Q prefixes, then the first q-block's mask, then the rest.
    g0w = GROUPS[0][1] * 128
    for half in range(2):
        hs = slice(64 * half, 64 * half + 64)
        nc.sync.dma_start(out=kT_sb[hs, 0, 0:g0w], in_=kT[0, :, 0:g0w])
        nc.sync.dma_start(out=qT_sb[hs, 0, 0:qb_size], in_=qT[0, :, 0:qb_size])
    mk0 = mask_pool.tile([128, N_CHUNKS, qb_size], F16, name="mk")
    mask_dma(mk0, slice(0, qb_size))
    for half in range(2):
        hs = slice(64 * half, 64 * half + 64)
        nc.sync.dma_start(out=kT_sb[hs, 0, g0w:], in_=kT[0, :, g0w:])
    for h in range(hpc):
        nc.sync.dma_start(
            out=vaug_sb[:, h, :, :],
            in_=vaug[h, :, :].rearrange("(c p) w -> p c w", p=128),
        )
    for half in range(2):
        hs = slice(64 * half, 64 * half + 64)
        nc.sync.dma_start(out=qT_sb[hs, 0, qb_size:], in_=qT[0, :, qb_size:])
        for h in range(1, hpc):
            nc.sync.dma_start(out=kT_sb[hs, h, :], in_=kT[h, :, :])
            nc.sync.dma_start(out=qT_sb[hs, h, :], in_=qT[h, :, :])

    # HAM warm-up: dummy fp16 matmuls spanning the input-DMA prologue.
    warm = const.tile([128, 512], F16)
    nc.vector.memset(warm, 0.0)
    wp = psum_s_pool.tile([128, GROUP_SIZE, qb_size], F32, name="wp", tag="ps")
    for _ in range(warm_mms):
        nc.tensor.matmul(wp[:, 0, :], lhsT=warm[:, 0:128], rhs=warm[:, :],
                         start=True, stop=True)

    def flush_avs(pending):
        for (h_, avt, qs_) in pending:
            osb = out_pool.tile([D + 1, qb_size], F32, name="osb")
            nc.vector.tensor_copy(osb[:, :], avt[:, :])
            nc.sync.dma_start(out=outT[h_, :, qs_], in_=osb[:, :])
        pending.clear()

    pending = []
    for qb in range(n_qb):
        qs = slice(qb * qb_size, (qb + 1) * qb_size)
        mk = mk0 if qb == 0 else mask_pool.tile(
            [128, N_CHUNKS, qb_size], F16, name="mk")
        if qb != 0:
            mask_dma(mk, qs)
        for h in range(hpc):
            av = psum_av_pool.tile([D + 1, qb_size], F32, tag="av", name="av")

            def emit_av(pt_t, c0, gs):
                for j in range(gs):
                    c = c0 + j
                    nc.tensor.matmul(
                        av[:, :],
                        lhsT=vaug_sb[:, h, c, :],
                        rhs=pt_t[:, j, :],
                        start=(c == 0),
                        stop=(c == N_CHUNKS - 1),
                    )

            par = 0
            prev_av = None  # AV lags QK by one group so the in-order PE
            # queue always holds ready work while the exp drain runs
            for gi, (c0, gs) in enumerate(GROUPS):
                if gi == 1 and h == 0:
                    flush_avs(pending)
                ps = psum_s_pool.tile([128, GROUP_SIZE, qb_size], F32,
                                      name="ps", tag="ps")
                for j in range(gs):
                    c = c0 + j
                    hs = slice(64 * par, 64 * par + 64)
                    nc.tensor.matmul(
                        ps[:, j, :],
                        lhsT=kT_sb[hs, h, c * 128:(c + 1) * 128],
                        rhs=qT_sb[hs, h, qs],
                        start=True,
                        stop=True,
                    )
                    par ^= 1
                mt = mk[:, c0:c0 + gs, :]
                pt = pt_pool.tile([128, GROUP_SIZE, qb_size], F16, name="pt")
                if gi in DVE_GROUPS:
                    nc.vector.scalar_tensor_tensor(
                        out=pt[:, :gs, :].bitcast(U16),
                        in0=ps[:, :gs, :],
                        scalar=A_STT,
                        in1=mt,
                        op0=mybir.AluOpType.mult,
                        op1=mybir.AluOpType.add,
                    )
                else:
                    nc.scalar.activation(
                        pt[:, :gs, :], ps[:, :gs, :],
                        mybir.ActivationFunctionType.Exp, scale=SCALE,
                    )
                    nc.vector.tensor_mul(pt[:, :gs, :], pt[:, :gs, :], mt)
                if prev_av is not None:
                    emit_av(*prev_av)
                prev_av = (pt, c0, gs)
            emit_av(*prev_av)
            pending.append((h, av, qs))
    flush_avs(pending)
    ctx.close()


def build_nc(s=S, hpc=HPC, **kwargs):
    nc = bacc.Bacc(
        "TRN2",
        target_bir_lowering=False,
        debug=False,
        num_devices=N_CORES,
    )
    qT = nc.dram_tensor("qT", [hpc, D, s], F16, kind="ExternalInput").ap()
    kT = nc.dram_tensor("kT", [hpc, D, s], F16, kind="ExternalInput").ap()
    vaug = nc.dram_tensor("vaug", [hpc, s, D + 1], F16, kind="ExternalInput").ap()
    maskT = nc.dram_tensor("maskT", [s, s], F16, kind="ExternalInput").ap()
    outT = nc.dram_tensor("outT", [hpc, D + 1, s], F32, kind="ExternalOutput").ap()
    with tile.TileContext(nc) as tc:
        build_kernel_body(tc, qT, kT, vaug, maskT, outT, s=s, hpc=hpc, **kwargs)
    nc.compile()
    return nc


_NC_CACHE = {}


def get_nc():
    if "nc" not in _NC_CACHE:
        _NC_CACHE["nc"] = build_nc()
    return _NC_CACHE["nc"]


def make_in_maps(query, key, value, self_attn_mask):
    """Host-side shard + layout prep. Returns list of 8 per-core input dicts."""
    q = np.asarray(query, dtype=np.float32)
    k = np.asarray(key, dtype=np.float32)
    v = np.asarray(value, dtype=np.float32)
    m = np.asarray(self_attn_mask)
    ones = np.ones((S, 1), np.float32)
    dve_rows = np.zeros((S, 1), bool)
    for gi in DVE_GROUPS:
        c0, gs = GROUPS[gi]
        dve_rows[c0 * 128:(c0 + gs) * 128] = True
    in_maps = []
    for core in range(N_CORES):
        flats = [HPC * core + i for i in range(HPC)]
        pairs = [(f // H, f % H) for f in flats]
        b = pairs[0][0]
        qT = np.ascontiguousarray(
            np.stack([q[b_, h_].T for b_, h_ in pairs])).astype(np.float16)
        kT = np.ascontiguousarray(
            np.stack([k[b_, h_].T for b_, h_ in pairs])).astype(np.float16)
        vaug = np.ascontiguousarray(
            np.stack([np.concatenate([v[b_, h_], ones], axis=1)
                      for b_, h_ in pairs])).astype(np.float16)
        keep = (~m[b, 0]).T  # [S_k, S_q] True = keep
        maskT = np.where(
            dve_rows,
            np.where(keep, np.float16(MB_KEEP), np.float16(MB_MASK)),
            np.where(keep, np.float16(1.0), np.float16(0.0)),
        ).astype(np.float16)
        in_maps.append({"qT": qT, "kT": kT, "vaug": vaug,
                        "maskT": np.ascontiguousarray(maskT)})
    return in_maps


def gather_output(results):
    out = np.empty((B, H, S, D), np.float32)
    for core, r in enumerate(results):
        oT = r["outT"].astype(np.float32)  # [HPC, 65, S]
        for i in range(HPC):
            f = HPC * core + i
            b_, h_ = f // H, f % H
            out[b_, h_] = (oT[i, :D, :] / oT[i, D:D + 1, :]).T
    return out


def kernel(query, key, value, self_attn_mask, trace=False, tmpdir=None):
    nc = get_nc()
    in_maps = make_in_maps(query, key, value, self_attn_mask)
    kwargs = {"tmpdir": tmpdir} if tmpdir else {}
    res = run_bass_kernel_spmd(nc, in_maps, core_ids=list(range(N_CORES)),
                               trace=trace, **kwargs)
    out = gather_output(res.results)
    if trace:
        kernel.last_result = res
    return out


# revision 18
# speedup vs baseline: 1.0494x; 1.0494x over previous
"""Masked attention (B=2, H=8, S=4096, D=64) on 8 Trainium2 NeuronCores.

Sharding: batch*head parallel. Core c owns flat heads {2c, 2c+1} (same batch
index b = c // 4 for both, so the [S, S] mask is shared by both heads of a
core).

v4 design (per core):

  - Q^T / K^T stored twice on the partition axis (rows 0:64 == rows 64:128,
    per head). Consecutive QK^T chunk-matmuls alternate which 64-row half
    they engage, so adjacent matmuls occupy disjoint PE row groups
    (tile_position (0,0)/(64,0) auto-derived) and stream CONCURRENTLY:
    2 chunks per 512-cycle stream slot -> ~2x QK throughput. LDWEIGHTS also
    pulls ahead freely across row groups.
  - Heads processed sequentially per q-block; the QK score pool is
    double-buffered (2 x 3 PSUM banks) so the exp drains overlap the next
    group's matmuls. The q-block's full mask slice [S_k, 512] fp16 is staged
    in SBUF once and shared by both heads.
  - Softmax exp+mask split across engines (ScalarE is otherwise the
    bottleneck: ACTIVATE is 1 elem/lane/cycle):
      * 8 of 11 k-groups: ScalarE ACT exp (PSUM->SBUF fp16) + VectorE
        tensor_mul with the {1,0} fp16 mask rows.
      * 3 of 11 k-groups: one VectorE scalar_tensor_tensor emitting
        saturate_uint16(round(score * (2^10*log2e/8) + mb)): the Schraudolph
        exp2 bit trick - those uint16 bits read back as fp16
        exp(score/8) * 2^(c/1024). c = -60 zeroes the mean (1+f)/2^f
        mismatch so these rows mix unbiased with exact-exp rows in the same
        softmax. Masked entries get mb = -60000 -> y < 0 -> saturates to
        0x0000 = +0.0. The mask tile doubles as the bias tile (host bakes
        the per-row-group encoding), so masking costs nothing extra here.
  - AV accumulated transposed-free: matmul(lhsT=[V|1] chunk [128,65],
    rhs=P^T chunk [128,512] fp16) accumulates out^T[d,q] over 32 k-chunks in
    PSUM; row 64 = softmax denominator; host divides + transposes during
    unshard.
"""

from contextlib import ExitStack

import numpy as np

import concourse.tile as tile
from concourse import bacc, mybir
from concourse.bass_utils import run_bass_kernel_spmd

B, H, S, D = 2, 8, 4096, 64
N_CORES = 8
HPC = (B * H) // N_CORES  # heads per core = 2
SCALE = 1.0 / 8.0  # 1/sqrt(D)
LOG2E = 1.4426950408889634
A_STT = float(np.float32(SCALE * 1024.0 * LOG2E))  # Schraudolph slope
MB_KEEP = 15 * 1024.0 - 60.0  # kept bias; c=-60 zeroes the mean 2^f/(1+f)
# mismatch so Schraudolph-path rows mix unbiased with exact-exp rows
MB_MASK = -60000.0  # masked: y always < 0 -> uint16 saturates to +0.0

F32 = mybir.dt.float32
F16 = mybir.dt.float16
U16 = mybir.dt.uint16

GROUP_SIZE = 3
N_CHUNKS = S // 128
GROUPS = []
_c0 = 0
while _c0 < N_CHUNKS:
    GROUPS.append((_c0, min(GROUP_SIZE, N_CHUNKS - _c0)))
    _c0 += GROUP_SIZE
# groups handled by the DVE Schraudolph path (rest go ScalarE exp).
DVE_GROUPS = (1, 5, 8)
MASK_DMA_CHUNKS = 8  # mask slice arrives in 4 DMAs of 8 k-chunks each


def build_kernel_body(tc, qT, kT, vaug, maskT, outT, s=S, hpc=HPC,
                      qb_size=512, pt_bufs=6, warm_mms=44):
    """qT, kT: [hpc, 64, s] f16; vaug: [hpc, s, 65] f16; maskT: [s, s] f16
    mixed-encoding; outT: [hpc, 65, s] f32."""
    nc = tc.nc
    n_qb = s // qb_size

    ctx = ExitStack()
    const = ctx.enter_context(tc.tile_pool(name="const", bufs=1))
    mask_pool = ctx.enter_context(tc.tile_pool(name="mask", bufs=2))
    pt_pool = ctx.enter_context(tc.tile_pool(name="pt", bufs=pt_bufs))
    out_pool = ctx.enter_context(tc.tile_pool(name="osb", bufs=2))
    psum_s_pool = ctx.enter_context(
        tc.tile_pool(name="psum_s", bufs=2, space="PSUM"))
    psum_av_pool = ctx.enter_context(
        tc.tile_pool(name="psum_av", bufs=hpc, space="PSUM"))

    qT_sb = const.tile([128, hpc, s], F16)
    kT_sb = const.tile([128, hpc, s], F16)
    vaug_sb = const.tile([128, hpc, N_CHUNKS, D + 1], F16)

    def mask_dma(mk_t, qs):
        for i in range(0, N_CHUNKS, MASK_DMA_CHUNKS):
            nc.sync.dma_start(
                out=mk_t[:, i:i + MASK_DMA_CHUNKS, :],
                in_=maskT[i * 128:(i + MASK_DMA_CHUNKS) * 128, qs].rearrange(
                    "(c p) q -> p c q", p=128
                ),
            )

    # Prologue DMAs, smallest-first so compute starts early: both dup halves
    # of head0's K/Q prefixes, then the first q-block's mask, then the rest.
    g0w = GROUPS[0][1] * 128
    for half in range(2):
        hs = slice(64 * half, 64 * half + 64)
        nc.sync.dma_start(out=kT_sb[hs, 0, 0:g0w], in_=kT[0, :, 0:g0w])
        nc.sync.dma_start(out=qT_sb[hs, 0, 0:qb_size], in_=qT[0, :, 0:qb_size])
    mk0 = mask_pool.tile([128, N_CHUNKS, qb_size], F16, name="mk")
    mask_dma(mk0, slice(0, qb_size))
    for half in range(2):
        hs = slice(64 * half, 64 * half + 64)
        nc.sync.dma_start(out=kT_sb[hs, 0, g0w:], in_=kT[0, :, g0w:])
    for h in range(hpc):
        nc.sync.dma_start(
            out=vaug_sb[:, h, :, :],
            in_=vaug[h, :, :].rearrange("(c p) w -> p c w", p=128),
        )
    for half in range(2):
        hs = slice(64 * half, 64 * half + 64)
        nc.sync.dma_start(out=qT_sb[hs, 0, qb_size:], in_=qT[0, :, qb_size:])
        for h in range(1, hpc):
            nc.sync.dma_start(out=kT_sb[hs, h, :], in_=kT[h, :, :])
            nc.sync.dma_start(out=qT_sb[hs, h, :], in_=qT[h, :, :])

    # HAM warm-up: dummy fp16 matmuls spanning the input-DMA prologue.
    warm = const.tile([128, 512], F16)
    nc.vector.memset(warm, 0.0)
    wp = psum_s_pool.tile([128, GROUP_SIZE, qb_size], F32, name="wp", tag="ps")
    for _ in range(warm_mms):
        nc.tensor.matmul(wp[:, 0, :], lhsT=warm[:, 0:128], rhs=warm[:, :],
                         start=True, stop=True)

    def flush_avs(pending):
        for (h_, avt, qs_) in pending:
            osb = out_pool.tile([D + 1, qb_size], F32, name="osb")
            nc.vector.tensor_copy(osb[:, :], avt[:, :])
            nc.sync.dma_start(out=outT[h_, :, qs_], in_=osb[:, :])
        pending.clear()

    pending = []
    for qb in range(n_qb):
        qs = slice(qb * qb_size, (qb + 1) * qb_size)
        mk = mk0 if qb == 0 else mask_pool.tile(
            [128, N_CHUNKS, qb_size], F16, name="mk")
        if qb != 0:
            mask_dma(mk, qs)
        for h in range(hpc):
            av = psum_av_pool.tile([D + 1, qb_size], F32, tag="av", name="av")

            def emit_av(pt_t, c0, gs):
                for j in range(gs):
                    c = c0 + j
                    nc.tensor.matmul(
                        av[:, :],
                        lhsT=vaug_sb[:, h, c, :],
                        rhs=pt_t[:, j, :],
                        start=(c == 0),
                        stop=(c == N_CHUNKS - 1),
                    )

            par = 0
            av_fifo = []  # AV lags QK by two groups so the in-order PE queue
            # always holds ready work while the exp drain + mask-mul run
            for gi, (c0, gs) in enumerate(GROUPS):
                if gi == 1 and h == 0:
                    flush_avs(pending)
                ps = psum_s_pool.tile([128, GROUP_SIZE, qb_size], F32,
                                      name="ps", tag="ps")
                for j in range(gs):
                    c = c0 + j
                    hs = slice(64 * par, 64 * par + 64)
                    nc.tensor.matmul(
                        ps[:, j, :],
                        lhsT=kT_sb[hs, h, c * 128:(c + 1) * 128],
                        rhs=qT_sb[hs, h, qs],
                        start=True,
                        stop=True,
                    )
                    par ^= 1
                mt = mk[:, c0:c0 + gs, :]
                pt = pt_pool.tile([128, GROUP_SIZE, qb_size], F16, name="pt")
                if gi in DVE_GROUPS:
                    nc.vector.scalar_tensor_tensor(
                        out=pt[:, :gs, :].bitcast(U16),
                        in0=ps[:, :gs, :],
                        scalar=A_STT,
                        in1=mt,
                        op0=mybir.AluOpType.mult,
                        op1=mybir.AluOpType.add,
                    )
                else:
                    nc.scalar.activation(
                        pt[:, :gs, :], ps[:, :gs, :],
                        mybir.ActivationFunctionType.Exp, scale=SCALE,
                    )
                    nc.vector.tensor_mul(pt[:, :gs, :], pt[:, :gs, :], mt)
                av_fifo.append((pt, c0, gs))
                if len(av_fifo) > 2:
                    emit_av(*av_fifo.pop(0))
            for item in av_fifo:
                emit_av(*item)
            pending.append((h, av, qs))
    flush_avs(pending)
    ctx.close()


def build_nc(s=S, hpc=HPC, **kwargs):
    nc = bacc.Bacc(
        "TRN2",
        target_bir_lowering=False,
        debug=False,
        num_devices=N_CORES,
    )
    qT = nc.dram_tensor("qT", [hpc, D, s], F16, kind="ExternalInput").ap()
    kT = nc.dram_tensor("kT", [hpc, D, s], F16, kind="ExternalInput").ap()
    vaug = nc.dram_tensor("vaug", [hpc, s, D + 1], F16, kind="ExternalInput").ap()
    maskT = nc.dram_tensor("maskT", [s, s], F16, kind="ExternalInput").ap()
    outT = nc.dram_tensor("outT", [hpc, D + 1, s], F32, kind="ExternalOutput").ap()
    with tile.TileContext(nc) as tc:
        build_kernel_body(tc, qT, kT, vaug, maskT, outT, s=s, hpc=hpc, **kwargs)
    nc.compile()
    return nc


_NC_CACHE = {}


def get_nc():
    if "nc" not in _NC_CACHE:
        _NC_CACHE["nc"] = build_nc()
    return _NC_CACHE["nc"]


def make_in_maps(query, key, value, self_attn_mask):
    """Host-side shard + layout prep. Returns list of 8 per-core input dicts."""
    q = np.asarray(query, dtype=np.float32)
    k = np.asarray(key, dtype=np.float32)
    v = np.asarray(value, dtype=np.float32)
    m = np.asarray(self_attn_mask)
    ones = np.ones((S, 1), np.float32)
    dve_rows = np.zeros((S, 1), bool)
    for gi in DVE_GROUPS:
        c0, gs = GROUPS[gi]
        dve_rows[c0 * 128:(c0 + gs) * 128] = True
    in_maps = []
    for core in range(N_CORES):
        flats = [HPC * core + i for i in range(HPC)]
        pairs = [(f // H, f % H) for f in flats]
        b = pairs[0][0]
        qT = np.ascontiguousarray(
            np.stack([q[b_, h_].T for b_, h_ in pairs])).astype(np.float16)
        kT = np.ascontiguousarray(
            np.stack([k[b_, h_].T for b_, h_ in pairs])).astype(np.float16)
        vaug = np.ascontiguousarray(
            np.stack([np.concatenate([v[b_, h_], ones], axis=1)
                      for b_, h_ in pairs])).astype(np.float16)
        keep = (~m[b, 0]).T  # [S_k, S_q] True = keep
        maskT = np.where(
            dve_rows,
            np.where(keep, np.float16(MB_KEEP), np.float16(MB_MASK)),
            np.where(keep, np.float16(1.0), np.float16(0.0)),
        ).astype(np.float16)
        in_maps.append({"qT": qT, "kT": kT, "vaug": vaug,
                        "maskT": np.ascontiguousarray(maskT)})
    return in_maps


def gather_output(results):
    out = np.empty((B, H, S, D), np.float32)
    for core, r in enumerate(results):
        oT = r["outT"].astype(np.float32)  # [HPC, 65, S]
        for i in range(HPC):
            f = HPC * core + i
            b_, h_ = f // H, f % H
            out[b_, h_] = (oT[i, :D, :] / oT[i, D:D + 1, :]).T
    return out


def kernel(query, key, value, self_attn_mask, trace=False, tmpdir=None):
    nc = get_nc()
    in_maps = make_in_maps(query, key, value, self_attn_mask)
    kwargs = {"tmpdir": tmpdir} if tmpdir else {}
    res = run_bass_kernel_spmd(nc, in_maps, core_ids=list(range(N_CORES)),
                               trace=trace, **kwargs)
    out = gather_output(res.results)
    if trace:
        kernel.last_result = res
    return out


# revision 19
# speedup vs baseline: 1.1712x; 1.1160x over previous
"""Masked attention (B=2, H=8, S=4096, D=64) on 8 Trainium2 NeuronCores.

Sharding: batch*head parallel. Core c owns flat heads {2c, 2c+1} (same batch
index b = c // 4 for both, so the [S, S] mask is shared by both heads of a
core).

v4 design (per core):

  - Q^T / K^T stored twice on the partition axis (rows 0:64 == rows 64:128,
    per head). Consecutive QK^T chunk-matmuls alternate which 64-row half
    they engage, so adjacent matmuls occupy disjoint PE row groups
    (tile_position (0,0)/(64,0) auto-derived) and stream CONCURRENTLY:
    2 chunks per 512-cycle stream slot -> ~2x QK throughput. LDWEIGHTS also
    pulls ahead freely across row groups.
  - Heads processed sequentially per q-block; the QK score pool is
    double-buffered (2 x 3 PSUM banks) so the exp drains overlap the next
    group's matmuls. The q-block's full mask slice [S_k, 512] fp16 is staged
    in SBUF once and shared by both heads.
  - Softmax exp+mask split across engines (ScalarE is otherwise the
    bottleneck: ACTIVATE is 1 elem/lane/cycle):
      * 8 of 11 k-groups: ScalarE ACT exp (PSUM->SBUF fp16) + VectorE
        tensor_mul with the {1,0} fp16 mask rows.
      * 3 of 11 k-groups: one VectorE scalar_tensor_tensor emitting
        saturate_uint16(round(score * (2^10*log2e/8) + mb)): the Schraudolph
        exp2 bit trick - those uint16 bits read back as fp16
        exp(score/8) * 2^(c/1024). c = -60 zeroes the mean (1+f)/2^f
        mismatch so these rows mix unbiased with exact-exp rows in the same
        softmax. Masked entries get mb = -60000 -> y < 0 -> saturates to
        0x0000 = +0.0. The mask tile doubles as the bias tile (host bakes
        the per-row-group encoding), so masking costs nothing extra here.
  - AV accumulated transposed-free: matmul(lhsT=[V|1] chunk [128,65],
    rhs=P^T chunk [128,512] fp16) accumulates out^T[d,q] over 32 k-chunks in
    PSUM; row 64 = softmax denominator; host divides + transposes during
    unshard.
"""

from contextlib import ExitStack

import numpy as np

import concourse.tile as tile
from concourse import bacc, mybir
from concourse.bass_utils import run_bass_kernel_spmd

B, H, S, D = 2, 8, 4096, 64
N_CORES = 8
HPC = (B * H) // N_CORES  # heads per core = 2
SCALE = 1.0 / 8.0  # 1/sqrt(D)
LOG2E = 1.4426950408889634
A_STT = float(np.float32(SCALE * 1024.0 * LOG2E))  # Schraudolph slope
MB_KEEP = 15 * 1024.0 - 60.0  # kept bias; c=-60 zeroes the mean 2^f/(1+f)
# mismatch so Schraudolph-path rows mix unbiased with exact-exp rows
MB_MASK = -60000.0  # masked: y always < 0 -> uint16 saturates to +0.0

F32 = mybir.dt.float32
F16 = mybir.dt.float16
U16 = mybir.dt.uint16

GROUP_SIZE = 2
N_CHUNKS = S // 128
GROUPS = []
_c0 = 0
while _c0 < N_CHUNKS:
    GROUPS.append((_c0, min(GROUP_SIZE, N_CHUNKS - _c0)))
    _c0 += GROUP_SIZE
# groups handled by the DVE Schraudolph path (rest go ScalarE exp).
DVE_GROUPS = (1, 4, 7, 10, 13)
MASK_DMA_CHUNKS = 8  # mask slice arrives in 4 DMAs of 8 k-chunks each


def build_kernel_body(tc, qT, kT, vaug, maskT, outT, s=S, hpc=HPC,
                      qb_size=512, pt_bufs=6, warm_mms=44):
    """qT, kT: [hpc, 64, s] f16; vaug: [hpc, s, 65] f16; maskT: [s, s] f16
    mixed-encoding; outT: [hpc, 65, s] f32."""
    nc = tc.nc
    n_qb = s // qb_size

    ctx = ExitStack()
    const = ctx.enter_context(tc.tile_pool(name="const", bufs=1))
    mask_pool = ctx.enter_context(tc.tile_pool(name="mask", bufs=2))
    pt_pool = ctx.enter_context(tc.tile_pool(name="pt", bufs=pt_bufs))
    out_pool = ctx.enter_context(tc.tile_pool(name="osb", bufs=2))
    psum_s_pool = ctx.enter_context(
        tc.tile_pool(name="psum_s", bufs=3, space="PSUM"))
    psum_av_pool = ctx.enter_context(
        tc.tile_pool(name="psum_av", bufs=hpc, space="PSUM"))

    qT_sb = const.tile([128, hpc, s], F16)
    kT_sb = const.tile([128, hpc, s], F16)
    vaug_sb = const.tile([128, hpc, N_CHUNKS, D + 1], F16)

    def mask_dma(mk_t, qs):
        for i in range(0, N_CHUNKS, MASK_DMA_CHUNKS):
            nc.sync.dma_start(
                out=mk_t[:, i:i + MASK_DMA_CHUNKS, :],
                in_=maskT[i * 128:(i + MASK_DMA_CHUNKS) * 128, qs].rearrange(
                    "(c p) q -> p c q", p=128
                ),
            )

    # Prologue DMAs, smallest-first so compute starts early: both dup halves
    # of head0's K/Q prefixes, then the first q-block's mask, then the rest.
    g0w = GROUPS[0][1] * 128
    for half in range(2):
        hs = slice(64 * half, 64 * half + 64)
        nc.sync.dma_start(out=kT_sb[hs, 0, 0:g0w], in_=kT[0, :, 0:g0w])
        nc.sync.dma_start(out=qT_sb[hs, 0, 0:qb_size], in_=qT[0, :, 0:qb_size])
    mk0 = mask_pool.tile([128, N_CHUNKS, qb_size], F16, name="mk")
    mask_dma(mk0, slice(0, qb_size))
    for half in range(2):
        hs = slice(64 * half, 64 * half + 64)
        nc.sync.dma_start(out=kT_sb[hs, 0, g0w:], in_=kT[0, :, g0w:])
    for h in range(hpc):
        nc.sync.dma_start(
            out=vaug_sb[:, h, :, :],
            in_=vaug[h, :, :].rearrange("(c p) w -> p c w", p=128),
        )
    for half in range(2):
        hs = slice(64 * half, 64 * half + 64)
        nc.sync.dma_start(out=qT_sb[hs, 0, qb_size:], in_=qT[0, :, qb_size:])
        for h in range(1, hpc):
            nc.sync.dma_start(out=kT_sb[hs, h, :], in_=kT[h, :, :])
            nc.sync.dma_start(out=qT_sb[hs, h, :], in_=qT[h, :, :])

    # HAM warm-up: dummy fp16 matmuls spanning the input-DMA prologue.
    warm = const.tile([128, 512], F16)
    nc.vector.memset(warm, 0.0)
    wp = psum_s_pool.tile([128, GROUP_SIZE, qb_size], F32, name="wp", tag="ps")
    for _ in range(warm_mms):
        nc.tensor.matmul(wp[:, 0, :], lhsT=warm[:, 0:128], rhs=warm[:, :],
                         start=True, stop=True)

    def flush_avs(pending):
        for (h_, avt, qs_) in pending:
            osb = out_pool.tile([D + 1, qb_size], F32, name="osb")
            nc.vector.tensor_copy(osb[:, :], avt[:, :])
            nc.sync.dma_start(out=outT[h_, :, qs_], in_=osb[:, :])
        pending.clear()

    pending = []
    for qb in range(n_qb):
        qs = slice(qb * qb_size, (qb + 1) * qb_size)
        mk = mk0 if qb == 0 else mask_pool.tile(
            [128, N_CHUNKS, qb_size], F16, name="mk")
        if qb != 0:
            mask_dma(mk, qs)
        for h in range(hpc):
            av = psum_av_pool.tile([D + 1, qb_size], F32, tag="av", name="av")

            def emit_av(pt_t, c0, gs):
                for j in range(gs):
                    c = c0 + j
                    nc.tensor.matmul(
                        av[:, :],
                        lhsT=vaug_sb[:, h, c, :],
                        rhs=pt_t[:, j, :],
                        start=(c == 0),
                        stop=(c == N_CHUNKS - 1),
                    )

            par = 0
            av_fifo = []  # AV lags QK by two groups so the in-order PE queue
            # always holds ready work while the exp drain + mask-mul run
            for gi, (c0, gs) in enumerate(GROUPS):
                if gi == 1 and h == 0:
                    flush_avs(pending)
                ps = psum_s_pool.tile([128, GROUP_SIZE, qb_size], F32,
                                      name="ps", tag="ps")
                for j in range(gs):
                    c = c0 + j
                    hs = slice(64 * par, 64 * par + 64)
                    nc.tensor.matmul(
                        ps[:, j, :],
                        lhsT=kT_sb[hs, h, c * 128:(c + 1) * 128],
                        rhs=qT_sb[hs, h, qs],
                        start=True,
                        stop=True,
                    )
                    par ^= 1
                mt = mk[:, c0:c0 + gs, :]
                pt = pt_pool.tile([128, GROUP_SIZE, qb_size], F16, name="pt")
                if gi in DVE_GROUPS:
                    nc.vector.scalar_tensor_tensor(
                        out=pt[:, :gs, :].bitcast(U16),
                        in0=ps[:, :gs, :],
                        scalar=A_STT,
                        in1=mt,
                        op0=mybir.AluOpType.mult,
                        op1=mybir.AluOpType.add,
                    )
                else:
                    nc.scalar.activation(
                        pt[:, :gs, :], ps[:, :gs, :],
                        mybir.ActivationFunctionType.Exp, scale=SCALE,
                    )
                    nc.vector.tensor_mul(pt[:, :gs, :], pt[:, :gs, :], mt)
                av_fifo.append((pt, c0, gs))
                if len(av_fifo) > 2:
                    emit_av(*av_fifo.pop(0))
            for item in av_fifo:
                emit_av(*item)
            pending.append((h, av, qs))
    flush_avs(pending)
    ctx.close()


def build_nc(s=S, hpc=HPC, **kwargs):
    nc = bacc.Bacc(
        "TRN2",
        target_bir_lowering=False,
        debug=False,
        num_devices=N_CORES,
    )
    qT = nc.dram_tensor("qT", [hpc, D, s], F16, kind="ExternalInput").ap()
    kT = nc.dram_tensor("kT", [hpc, D, s], F16, kind="ExternalInput").ap()
    vaug = nc.dram_tensor("vaug", [hpc, s, D + 1], F16, kind="ExternalInput").ap()
    maskT = nc.dram_tensor("maskT", [s, s], F16, kind="ExternalInput").ap()
    outT = nc.dram_tensor("outT", [hpc, D + 1, s], F32, kind="ExternalOutput").ap()
    with tile.TileContext(nc) as tc:
        build_kernel_body(tc, qT, kT, vaug, maskT, outT, s=s, hpc=hpc, **kwargs)
    nc.compile()
    return nc


_NC_CACHE = {}


def get_nc():
    if "nc" not in _NC_CACHE:
        _NC_CACHE["nc"] = build_nc()
    return _NC_CACHE["nc"]


def make_in_maps(query, key, value, self_attn_mask):
    """Host-side shard + layout prep. Returns list of 8 per-core input dicts."""
    q = np.asarray(query, dtype=np.float32)
    k = np.asarray(key, dtype=np.float32)
    v = np.asarray(value, dtype=np.float32)
    m = np.asarray(self_attn_mask)
    ones = np.ones((S, 1), np.float32)
    dve_rows = np.zeros((S, 1), bool)
    for gi in DVE_GROUPS:
        c0, gs = GROUPS[gi]
        dve_rows[c0 * 128:(c0 + gs) * 128] = True
    in_maps = []
    for core in range(N_CORES):
        flats = [HPC * core + i for i in range(HPC)]
        pairs = [(f // H, f % H) for f in flats]
        b = pairs[0][0]
        qT = np.ascontiguousarray(
            np.stack([q[b_, h_].T for b_, h_ in pairs])).astype(np.float16)
        kT = np.ascontiguousarray(
            np.stack([k[b_, h_].T for b_, h_ in pairs])).astype(np.float16)
        vaug = np.ascontiguousarray(
            np.stack([np.concatenate([v[b_, h_], ones], axis=1)
                      for b_, h_ in pairs])).astype(np.float16)
        keep = (~m[b, 0]).T  # [S_k, S_q] True = keep
        maskT = np.where(
            dve_rows,
            np.where(keep, np.float16(MB_KEEP), np.float16(MB_MASK)),
            np.where(keep, np.float16(1.0), np.float16(0.0)),
        ).astype(np.float16)
        in_maps.append({"qT": qT, "kT": kT, "vaug": vaug,
                        "maskT": np.ascontiguousarray(maskT)})
    return in_maps


def gather_output(results):
    out = np.empty((B, H, S, D), np.float32)
    for core, r in enumerate(results):
        oT = r["outT"].astype(np.float32)  # [HPC, 65, S]
        for i in range(HPC):
            f = HPC * core + i
            b_, h_ = f // H, f % H
            out[b_, h_] = (oT[i, :D, :] / oT[i, D:D + 1, :]).T
    return out


def kernel(query, key, value, self_attn_mask, trace=False, tmpdir=None):
    nc = get_nc()
    in_maps = make_in_maps(query, key, value, self_attn_mask)
    kwargs = {"tmpdir": tmpdir} if tmpdir else {}
    res = run_bass_kernel_spmd(nc, in_maps, core_ids=list(range(N_CORES)),
                               trace=trace, **kwargs)
    out = gather_output(res.results)
    if trace:
        kernel.last_result = res
    return out
